# revision 12
# baseline (speedup 1.0000x reference)
"""Trainium2 Bass kernel for nn_CGPBlock (attention block with 1x1-conv QKV).

Reference computation (per batch b):
    q = Wq @ pose + bq; k = Wk @ id + bk; v = Wv @ pose + bv     # [C, L]
    energy[i, j] = sum_c q[c, i] k[c, j]                          # [L, L]
    attn = softmax_j(energy)
    va[c, i] = sum_j v[c, j] attn[i, j]
    out = pose + gamma * va
Sharding: data-parallel over batch, B=8 batches -> 8 NeuronCores (SPMD).

Device algorithm (per core, matmuls bf16 with fp32 PSUM accumulate):
  - chunk-0 convs run up front (full-width PSUM pool, closed before the
    attention pools open); chunks 1-3 convs are interleaved INTO the
    attention j-loop as one-bank [C,512] halves so the exp/matmul pipeline
    starts ~15us earlier and the PE never idles long enough for the HAM
    clock gate to re-throttle.
  - For each i-chunk (1024 cols), 32 j-tiles:
      eT[j, i] = k_jt.T @ q_chunk              (PSUM, 2 x N=512 matmuls)
      pT = exp(eT): ACT spline exp for most tiles; for 7/32 tiles a
           one-op DVE Schraudolph fast-exp (affine -> int16, bitcast bf16)
           offloads the saturated ACT engine (|E|<33 so no max-sub needed;
           softmax's correlated num/denom error cancellation keeps the
           ~3% fast-exp noise at ~3e-4 in the output)
      va[c, i] += vt_jt.T @ pT                 (PSUM accumulate)
  - Z (softmax denominators): DVE pre-sum tree pt->quad->s8->s16, then a
    per-chunk burst of M=1 matmuls in 512-wide halves through a single
    PSUM bank (serialized via pool reuse), emitted into the next chunk's
    j-loop so the PE burst and the reciprocal/broadcast latency hide
    under the next chunk's compute.
  - normalize/out: 1/Z broadcast across partitions via a DRAM round-trip
    per half; va*(1/Z)*gamma + pose' on gpsimd (DVE is near-saturated by
    the sum tree); pose' = pose + gamma*bv precomputed (attention rows
    sum to 1 so v's bias folds into the residual).
  - last chunk: no next-chunk compute to hide under -- Z's final quad
    skips the DVE tree (direct M=1 matmuls on the pt tiles), 1/Z is
    broadcast with PE matmuls, and the normalize drains in 512-wide
    half-pipelined steps.

Scheduling notes (Tile executes each engine's stream in program order):
  - va/Z matmuls are emitted SKEW j-tiles behind the energy matmuls so the
    PE never waits on the exp; the skew decays near the end of the last
    chunk so the PE tail drains right behind the final exps.
  - Every chunked tensor is a separate tile per chunk (Tile tracks
    dependencies at tile granularity).
  - ~8 garbage matmuls pre-warm the PE clock gate (HAM) during the input
    DMAs so the chunk-0 convs run at 2.4 GHz.
  - PSUM banks: et 2x2 + va 2 + z 1 + conv-interleave 1 = 8/8.
"""

import numpy as np
import ml_dtypes

import concourse.bacc as bacc
import concourse.tile as tile
from concourse import mybir
from concourse.bass_utils import run_bass_kernel_spmd

F32 = mybir.dt.float32
BF16 = mybir.dt.bfloat16
I16 = mybir.dt.int16
AF = mybir.ActivationFunctionType
ALU = mybir.AluOpType

B, C, L = 8, 128, 4096
CHUNK = 1024                # i-chunk width
NCH = L // CHUNK            # 4 chunks
NJT = L // 128              # 32 j-tiles
JPC = CHUNK // 128          # j-tiles per chunk tile
QUAD = 4                    # j-tiles per DVE quad-sum
SKEW = 8                    # software pipeline depth (PE runs ahead of ACT)
WARM = 8                    # PE pre-warm matmuls (cover the first DMA wait)

# Schraudolph fast-exp on DVE: exp(x) ~= bitcast_bf16(int16(x*SC_S + SC_B)).
SC_S = 128.0 / float(np.log(2.0))      # 2^7 * log2(e)
SC_B = 127.0 * 128.0 - 4.0             # exponent bias - centering constant


def _use_dve_exp(jt):
    # 7 of 32 exp tiles per chunk go to DVE
    return (jt % 8 == 2) or (jt % 8 == 6 and jt < 24)


_CACHE = {}


def _build():
    nc = bacc.Bacc("TRN2", target_bir_lowering=False, debug=False, num_devices=B)

    pose_d = nc.dram_tensor("pose", [C, L], F32, kind="ExternalInput").ap()
    posebf_d = nc.dram_tensor("posebf", [C, L], BF16, kind="ExternalInput").ap()
    idbf_d = nc.dram_tensor("idbf", [C, L], BF16, kind="ExternalInput").ap()
    wt_d = nc.dram_tensor("wt", [C, 3 * C], BF16, kind="ExternalInput").ap()
    bq_d = nc.dram_tensor("bq", [C, 1], F32, kind="ExternalInput").ap()
    bk_d = nc.dram_tensor("bk", [C, 1], F32, kind="ExternalInput").ap()
    bfin_d = nc.dram_tensor("bfin", [C, 1], F32, kind="ExternalInput").ap()
    gam_d = nc.dram_tensor("gam", [C, 1], F32, kind="ExternalInput").ap()
    out_d = nc.dram_tensor("out", [C, L], F32, kind="ExternalOutput").ap()

    with tile.TileContext(nc) as tc:
        with tc.tile_pool(name="res", bufs=1) as res:
            wt_sb = res.tile([C, 3 * C], BF16)
            nc.sync.dma_start(wt_sb, wt_d)
            bq_sb = res.tile([C, 1], F32)
            bk_sb = res.tile([C, 1], F32)
            nc.sync.dma_start(bq_sb, bq_d)
            nc.sync.dma_start(bk_sb, bk_d)
            bfin_sb = res.tile([C, 1], F32)
            nc.gpsimd.dma_start(bfin_sb, bfin_d)
            gam_sb = res.tile([C, 1], F32)
            nc.gpsimd.dma_start(gam_sb, gam_d)
            ones_sb = res.tile([C, 1], BF16)
            nc.vector.memset(ones_sb, 1.0)
            onesr_sb = res.tile([1, C], F32)
            nc.vector.memset(onesr_sb, 1.0)

            def chunk_tiles(prefix, dtype):
                return [res.tile([C, CHUNK], dtype, name=f"{prefix}{i}")
                        for i in range(NCH)]

            pose_t = chunk_tiles("pose", F32)
            posebf_t = chunk_tiles("posebf", BF16)
            idbf_t = chunk_tiles("idbf", BF16)
            q_t = chunk_tiles("q", BF16)
            k_t = chunk_tiles("k", BF16)
            v_t = chunk_tiles("v", BF16)
            vt_t = chunk_tiles("vt", BF16)   # [j (partition), jt*128 + c]

            # chunk-0 inputs first: only they gate the pipeline start
            for ch in range(NCH):
                sl = slice(ch * CHUNK, (ch + 1) * CHUNK)
                nc.sync.dma_start(idbf_t[ch], idbf_d[:, sl])
                nc.sync.dma_start(posebf_t[ch], posebf_d[:, sl])
            for ch in range(NCH):
                sl = slice(ch * CHUNK, (ch + 1) * CHUNK)
                nc.gpsimd.dma_start(pose_t[ch], pose_d[:, sl])

            wqT = wt_sb[:, 0:C]
            wkT = wt_sb[:, C:2 * C]
            wvT = wt_sb[:, 2 * C:3 * C]

            # PE clock-gate pre-warm (no DMA deps)
            warm_sb = res.tile([C, 512], BF16)
            nc.vector.memset(warm_sb, 0.0)
            with tc.tile_pool(name="warm_ps", bufs=1, space="PSUM") as warm_ps:
                wp = warm_ps.tile([1, 512], F32)
                for _ in range(WARM):
                    nc.tensor.matmul(wp, lhsT=ones_sb, rhs=warm_sb,
                                     start=True, stop=True)

            # ---- QKV convs, all chunks upfront ----
            with tc.tile_pool(name="conv_ps", bufs=3, space="PSUM") as conv_ps:
                for cch in range(NCH):
                    kp = conv_ps.tile([C, CHUNK], F32, tag="cv", name="kp")
                    qp = conv_ps.tile([C, CHUNK], F32, tag="cv", name="qp")
                    vp = conv_ps.tile([C, CHUNK], F32, tag="cv", name="vp")
                    for h in range(CHUNK // 512):
                        hs = slice(h * 512, (h + 1) * 512)
                        nc.tensor.matmul(kp[:, hs], lhsT=wkT,
                                         rhs=idbf_t[cch][:, hs], start=True, stop=True)
                        nc.tensor.matmul(qp[:, hs], lhsT=wqT,
                                         rhs=posebf_t[cch][:, hs], start=True, stop=True)
                        nc.tensor.matmul(vp[:, hs], lhsT=wvT,
                                         rhs=posebf_t[cch][:, hs], start=True, stop=True)
                    nc.scalar.activation(k_t[cch], kp, AF.Identity, bias=bk_sb)
                    nc.vector.tensor_scalar_add(q_t[cch], qp, bq_sb)
                    nc.scalar.copy(v_t[cch], vp)
                    nc.sync.dma_start_transpose(
                        vt_t[cch].rearrange("p (t c) -> p t c", c=C), v_t[cch])

            # ---- attention (convs for ch 1-3 interleaved into the j-loop) ----
            with (
                tc.tile_pool(name="et_ps", bufs=2, space="PSUM") as et_ps,
                tc.tile_pool(name="va_ps", bufs=1, space="PSUM") as va_ps,
                tc.tile_pool(name="z_ps", bufs=1, space="PSUM") as z_ps,
                tc.tile_pool(name="ci_ps", bufs=1, space="PSUM") as ci_ps,
                tc.tile_pool(name="pt_sb", bufs=SKEW + QUAD + 2) as pt_pool,
                tc.tile_pool(name="qs_sb", bufs=3) as qs_pool,
                tc.tile_pool(name="s16_sb", bufs=2) as s16_pool,
                tc.tile_pool(name="nrm", bufs=2) as nrm,
                tc.tile_pool(name="outb", bufs=2) as outb,
                tc.tile_pool(name="dramp", bufs=4, space="DRAM") as dramp,
            ):
                # conv-interleave jobs: (part, half) emitted at fixed jt slots
                CONV_JOBS = {3: ("k", 0), 5: ("k", 1), 7: ("q", 0), 9: ("q", 1),
                             11: ("v", 0), 13: ("v", 1)}

                def emit_conv_half(ch, part, h):
                    cp = ci_ps.tile([C, 512], F32)
                    hs = slice(h * 512, (h + 1) * 512)
                    if part == "k":
                        nc.tensor.matmul(cp, lhsT=wkT, rhs=idbf_t[ch][:, hs],
                                         start=True, stop=True)
                        nc.scalar.activation(k_t[ch][:, hs], cp, AF.Identity,
                                             bias=bk_sb)
                    elif part == "q":
                        nc.tensor.matmul(cp, lhsT=wqT, rhs=posebf_t[ch][:, hs],
                                         start=True, stop=True)
                        nc.vector.tensor_scalar_add(q_t[ch][:, hs], cp, bq_sb)
                    else:
                        nc.tensor.matmul(cp, lhsT=wvT, rhs=posebf_t[ch][:, hs],
                                         start=True, stop=True)
                        nc.scalar.copy(v_t[ch][:, hs], cp)

                # per-chunk Z bursts + normalize, emitted into the NEXT
                # chunk's j-loop (latency hides under its compute)
                def emit_z_half(z_src, h, zt):
                    hs = slice(h * 512, (h + 1) * 512)
                    for n, (kind, t) in enumerate(z_src):
                        nc.tensor.matmul(zt, lhsT=ones_sb, rhs=t[:, hs],
                                         start=(n == 0),
                                         stop=(n == len(z_src) - 1))

                prev = {}   # state of the previous chunk's normalize

                def emit_prev_step(step):
                    # steps injected at jt 0..5 of the following chunk
                    if not prev:
                        return
                    ch = prev["ch"]
                    if step == 0:
                        prev["zt0"] = z_ps.tile([1, 512], F32, tag="z", name="z0")
                        emit_z_half(prev["z_src"], 0, prev["zt0"])
                    elif step == 1:
                        rz = prev["rz"] = nrm.tile([1, CHUNK], F32, tag="rz", name="rz")
                        nc.vector.reciprocal_approx_fast(rz[0:1, 0:512],
                                                         prev["zt0"])
                        nc.vector.tensor_scalar_mul(rz[0:1, 0:512],
                                                    rz[0:1, 0:512],
                                                    gam_sb[0:1, :])
                        zd = dramp.tile([1, 512], F32)
                        nc.sync.dma_start(zd, rz[0:1, 0:512])
                        rzb = prev["rzb"] = nrm.tile([C, CHUNK], F32, tag="rzb", name="rzb")
                        nc.sync.dma_start(rzb[:, 0:512],
                                          zd.to_broadcast([C, 512]))
                    elif step == 2:
                        prev["zt1"] = z_ps.tile([1, 512], F32, tag="z", name="z1")
                        emit_z_half(prev["z_src"], 1, prev["zt1"])
                    elif step == 3:
                        rz, rzb = prev["rz"], prev["rzb"]
                        nc.vector.reciprocal_approx_fast(rz[0:1, 512:1024],
                                                         prev["zt1"])
                        nc.vector.tensor_scalar_mul(rz[0:1, 512:1024],
                                                    rz[0:1, 512:1024],
                                                    gam_sb[0:1, :])
                        zd = dramp.tile([1, 512], F32)
                        nc.sync.dma_start(zd, rz[0:1, 512:1024])
                        nc.sync.dma_start(rzb[:, 512:1024],
                                          zd.to_broadcast([C, 512]))
                    elif step == 4:
                        t = prev["t"] = nrm.tile([C, CHUNK], F32, tag="t", name="t")
                        nc.vector.tensor_mul(t, prev["va_sb"], prev["rzb"])
                        o = prev["o"] = outb.tile([C, CHUNK], F32, name="o")
                        nc.vector.tensor_add(o, t, pose_t[ch])
                    elif step == 5:
                        isl = slice(ch * CHUNK, (ch + 1) * CHUNK)
                        nc.sync.dma_start(out_d[:, isl], prev["o"])
                        prev.clear()

                for ch in range(NCH):
                    i0 = ch * CHUNK
                    last_ch = ch == NCH - 1
                    va = va_ps.tile([C, CHUNK], F32)
                    # pose' = pose + gamma*bv (gpsimd; needed at normalize)
                    nc.vector.tensor_scalar_add(pose_t[ch], pose_t[ch], bfin_sb)

                    pts = {}
                    sabs = {}
                    s8s = {}
                    s16s = {}

                    def skew_at(j):
                        if not last_ch:
                            return SKEW
                        return max(2, min(SKEW, NJT + 2 - j))

                    lag_ptr = 0
                    n_steps = NJT + 2 if last_ch else NJT + SKEW
                    for jt in range(n_steps):
                        emit_prev_step(jt)

                        if jt < NJT:
                            ksl = slice((jt % JPC) * 128, (jt % JPC + 1) * 128)
                            et = et_ps.tile([C, CHUNK], F32, tag="et", name="et")
                            for h in range(CHUNK // 512):
                                hs = slice(h * 512, (h + 1) * 512)
                                nc.tensor.matmul(
                                    et[:, hs], lhsT=k_t[jt // JPC][:, ksl],
                                    rhs=q_t[ch][:, hs], start=True, stop=True)
                            pt = pt_pool.tile([C, CHUNK], BF16)
                            if _use_dve_exp(jt):
                                nc.vector.tensor_scalar(
                                    pt.bitcast(I16), et,
                                    SC_S, SC_B, ALU.mult, ALU.add)
                            else:
                                nc.scalar.activation(pt, et, AF.Exp)
                            pts[jt] = pt

                        while lag_ptr <= min(jt - skew_at(jt), NJT - 1):
                            lag = lag_ptr
                            lag_ptr += 1
                            vsl = slice((lag % JPC) * 128, (lag % JPC + 1) * 128)
                            pt = pts[lag]
                            for h in range(CHUNK // 512):
                                hs = slice(h * 512, (h + 1) * 512)
                                nc.tensor.matmul(
                                    va[:, hs], lhsT=vt_t[lag // JPC][:, vsl],
                                    rhs=pt[:, hs],
                                    start=(lag == 0), stop=(lag == NJT - 1))
                            # DVE pre-sum tree: quad -> s8 -> s16.
                            # Last chunk: final quad skips the tree (its pt
                            # tiles feed the Z burst directly).
                            if last_ch and lag >= NJT - QUAD:
                                continue
                            if lag % QUAD == QUAD - 1:
                                qd = lag // QUAD
                                p0, p1, p2, p3 = (pts.pop(lag - 3),
                                                  pts.pop(lag - 2),
                                                  pts.pop(lag - 1),
                                                  pts.pop(lag))
                                sa = qs_pool.tile([C, CHUNK], BF16, tag="sa")
                                nc.vector.tensor_add(sa, p0, p1)
                                sb_ = qs_pool.tile([C, CHUNK], BF16, tag="sb")
                                nc.vector.tensor_add(sb_, p2, p3)
                                sab = qs_pool.tile([C, CHUNK], BF16, tag="sab")
                                nc.vector.tensor_add(sab, sa, sb_)
                                sabs[qd] = sab
                                if qd % 2 == 1:
                                    s8 = qs_pool.tile([C, CHUNK], BF16,
                                                      tag="s8")
                                    nc.vector.tensor_add(
                                        s8, sabs.pop(qd - 1), sabs.pop(qd))
                                    s8s[qd // 2] = s8
                                    if qd % 4 == 3:
                                        s16 = s16_pool.tile([C, CHUNK], BF16)
                                        nc.vector.tensor_add(
                                            s16, s8s.pop(qd // 2 - 1),
                                            s8s.pop(qd // 2))
                                        s16s[qd // 4] = s16

                    # ---- end of chunk j-loop ----
                    va_sb = nrm.tile([C, CHUNK], F32, tag="va_sb")
                    nc.scalar.copy(va_sb, va)

                    if not last_ch:
                        # z sources: two s16 tiles; burst emitted into the
                        # next chunk's loop via emit_prev_step
                        prev.clear()
                        prev.update({
                            "ch": ch, "va_sb": va_sb,
                            "z_src": [("t", s16s[0]), ("t", s16s[1])],
                        })
                    else:
                        # tail: z from {s16a, s8_2, sab6, pt28..31}, halves
                        # serialized; 1/Z broadcast on the (now idle) PE
                        z_src = [("t", s16s[0]), ("t", s8s[2]),
                                 ("t", sabs[6])]
                        z_src += [("t", pts[NJT - QUAD + i])
                                  for i in range(QUAD)]
                        rz = nrm.tile([1, CHUNK], F32, tag="rz")
                        rzb = et_ps.tile([C, CHUNK], F32, tag="et", name="rzb_ps")
                        t = nrm.tile([C, CHUNK], F32, tag="t")
                        o = outb.tile([C, CHUNK], F32)
                        for h in range(CHUNK // 512):
                            hs = slice(h * 512, (h + 1) * 512)
                            ihs = slice(i0 + h * 512, i0 + (h + 1) * 512)
                            zt = z_ps.tile([1, 512], F32, tag="z", name=f"zt{h}")
                            emit_z_half(z_src, h, zt)
                            nc.vector.reciprocal_approx_fast(rz[0:1, hs], zt)
                            nc.vector.tensor_scalar_mul(rz[0:1, hs],
                                                        rz[0:1, hs],
                                                        gam_sb[0:1, :])
                            nc.tensor.matmul(rzb[:, hs], lhsT=onesr_sb,
                                             rhs=rz[0:1, hs],
                                             start=True, stop=True)
                            nc.vector.tensor_mul(t[:, hs], va_sb[:, hs],
                                                 rzb[:, hs])
                            nc.vector.tensor_add(o[:, hs], t[:, hs],
                                                 pose_t[ch][:, hs])
                            nc.sync.dma_start(out_d[:, ihs], o[:, hs])

                # flush the final non-last-chunk normalize if pending
                for step in range(6):
                    emit_prev_step(step)

    nc.compile()
    return nc


def _get_nc():
    if "nc" not in _CACHE:
        _CACHE["nc"] = _build()
    return _CACHE["nc"]


def kernel(pose_f, id_f, Wq, bq, Wk, bk, Wv, bv, gamma, **run_kwargs):
    pose_f = np.asarray(pose_f, dtype=np.float32)
    id_f = np.asarray(id_f, dtype=np.float32)
    Wq = np.asarray(Wq, dtype=np.float32)
    Wk = np.asarray(Wk, dtype=np.float32)
    Wv = np.asarray(Wv, dtype=np.float32)
    bq = np.asarray(bq, dtype=np.float32)
    bk = np.asarray(bk, dtype=np.float32)
    bv = np.asarray(bv, dtype=np.float32)
    g = float(np.asarray(gamma, dtype=np.float32).reshape(-1)[0])

    bf = ml_dtypes.bfloat16
    wt = np.concatenate([Wq.T, Wk.T, Wv.T], axis=1).astype(bf)  # [C_in, 3C]
    posebf = pose_f.astype(bf)
    idbf = id_f.astype(bf)
    bq_c = np.ascontiguousarray(bq.reshape(C, 1))
    bk_c = np.ascontiguousarray(bk.reshape(C, 1))
    bfin = np.ascontiguousarray((g * bv).reshape(C, 1).astype(np.float32))
    gam = np.full((C, 1), g, dtype=np.float32)

    in_maps = []
    for b in range(B):
        in_maps.append({
            "pose": pose_f[b],
            "posebf": posebf[b],
            "idbf": idbf[b],
            "wt": wt,
            "bq": bq_c,
            "bk": bk_c,
            "bfin": bfin,
            "gam": gam,
        })

    nc = _get_nc()
    res = run_bass_kernel_spmd(nc, in_maps, core_ids=list(range(B)), **run_kwargs)
    out = np.stack([res.results[b]["out"] for b in range(B)], axis=0)
    if run_kwargs:
        _CACHE["last_result"] = res
    return out


# revision 13
# speedup vs baseline: 1.1228x; 1.1228x over previous
"""Trainium2 Bass kernel for nn_CGPBlock (attention block with 1x1-conv QKV).

Reference computation (per batch b):
    q = Wq @ pose + bq; k = Wk @ id + bk; v = Wv @ pose + bv     # [C, L]
    energy[i, j] = sum_c q[c, i] k[c, j]                          # [L, L]
    attn = softmax_j(energy)
    va[c, i] = sum_j v[c, j] attn[i, j]
    out = pose + gamma * va
Sharding: data-parallel over batch, B=8 batches -> 8 NeuronCores (SPMD).

Device algorithm (per core, matmuls bf16 with fp32 PSUM accumulate):
  - chunk-0's QKV convs run up front (own PSUM pool, closed before the
    attention pools open).  The convs for j/i-chunks 1-3 are interleaved
    INTO chunk-0's attention j-loop: each steals one tile of the energy
    PSUM rotation (2 x N=512 matmuls + one full-width drain), scheduled
    so k_t[n] lands before chunk-0's energy matmuls reach j-tile 8n and
    vt_t[n] (DMA xbar transpose) before the lagged va matmuls need it.
    This removes the serial [all convs]->[all drains]->[attention] head
    (~15us) and keeps the PE dense so the HAM clock gate stays at 8/8.
  - For each i-chunk (1024 cols), 32 j-tiles:
      eT[j, i] = k_jt.T @ q_chunk              (PSUM, 2 x N=512 matmuls)
      pT = exp(eT): ACT spline exp for most tiles; for 7/32 tiles a
           one-op DVE Schraudolph fast-exp (affine -> int16, bitcast bf16)
           offloads the saturated ACT engine (|E|<33 so no max-sub needed;
           softmax's correlated num/denom error cancellation keeps the
           ~3% fast-exp noise at ~3e-4 in the output)
      va[c, i] += vt_jt.T @ pT                 (PSUM accumulate)
      Z[1, i]  += ones.T @ (pT octsums)        (DVE pre-sums 8 j-tiles per
                                                M=1 matmul)
  - out = pose' + va * (gamma/Z) with gamma folded into the tiny 1/Z row
    before broadcasting it across partitions via a DRAM round-trip
    (latency hidden under the next chunk); pose' = pose + gamma*bv
    (attention rows sum to 1 so v's bias folds into the residual).
  - last chunk: Z's final quad skips the DVE tree (direct M=1 matmuls),
    1/Z is broadcast with PE matmuls (nothing left to hide a DMA under),
    and the normalize drains in 512-wide half-pipelined steps.

Scheduling notes (Tile executes each engine's stream in program order):
  - va/Z matmuls are emitted SKEW j-tiles behind the energy matmuls so the
    PE never waits on the exp; the skew decays near the end of the last
    chunk so the PE tail drains right behind the final exps.
  - ~6 garbage matmuls pre-warm the PE clock gate (HAM) during the input
    DMAs so the chunk-0 convs run at 2.4 GHz.
  - PSUM banks: et 2x2 (shared with interleaved convs + last-chunk 1/Z
    broadcast) + va 2 + z 2 = 8/8.
"""

import numpy as np
import ml_dtypes

import concourse.bacc as bacc
import concourse.tile as tile
from concourse import mybir
from concourse.bass_utils import run_bass_kernel_spmd

F32 = mybir.dt.float32
BF16 = mybir.dt.bfloat16
I16 = mybir.dt.int16
AF = mybir.ActivationFunctionType
ALU = mybir.AluOpType

B, C, L = 8, 128, 4096
CHUNK = 1024                # i-chunk width
NCH = L // CHUNK            # 4 chunks
NJT = L // 128              # 32 j-tiles
JPC = CHUNK // 128          # j-tiles per chunk tile
QUAD = 4                    # j-tiles pre-summed per Z matmul
SKEW = 8                    # software pipeline depth (PE runs ahead of ACT)
WARM = 6                    # PE pre-warm matmuls (cover the first DMA wait)

# Schraudolph fast-exp on DVE: exp(x) ~= bitcast_bf16(int16(x*SC_S + SC_B)).
SC_S = 128.0 / float(np.log(2.0))      # 2^7 * log2(e)
SC_B = 127.0 * 128.0 - 4.0             # exponent bias - centering constant


def _use_dve_exp(jt):
    # 7 of 32 exp tiles per chunk go to DVE
    return (jt % 8 == 2) or (jt % 8 == 6 and jt < 24)


_CACHE = {}


def _build():
    nc = bacc.Bacc("TRN2", target_bir_lowering=False, debug=False, num_devices=B)

    pose_d = nc.dram_tensor("pose", [C, L], F32, kind="ExternalInput").ap()
    posebf_d = nc.dram_tensor("posebf", [C, L], BF16, kind="ExternalInput").ap()
    idbf_d = nc.dram_tensor("idbf", [C, L], BF16, kind="ExternalInput").ap()
    wt_d = nc.dram_tensor("wt", [C, 3 * C], BF16, kind="ExternalInput").ap()
    bq_d = nc.dram_tensor("bq", [C, 1], F32, kind="ExternalInput").ap()
    bk_d = nc.dram_tensor("bk", [C, 1], F32, kind="ExternalInput").ap()
    bfin_d = nc.dram_tensor("bfin", [C, 1], F32, kind="ExternalInput").ap()
    gam_d = nc.dram_tensor("gam", [C, 1], F32, kind="ExternalInput").ap()
    out_d = nc.dram_tensor("out", [C, L], F32, kind="ExternalOutput").ap()

    with tile.TileContext(nc) as tc:
        with tc.tile_pool(name="res", bufs=1) as res:
            wt_sb = res.tile([C, 3 * C], BF16)
            nc.sync.dma_start(wt_sb, wt_d)
            bq_sb = res.tile([C, 1], F32)
            bk_sb = res.tile([C, 1], F32)
            nc.sync.dma_start(bq_sb, bq_d)
            nc.sync.dma_start(bk_sb, bk_d)
            bfin_sb = res.tile([C, 1], F32)
            nc.gpsimd.dma_start(bfin_sb, bfin_d)
            gam_sb = res.tile([C, 1], F32)
            nc.gpsimd.dma_start(gam_sb, gam_d)
            ones_sb = res.tile([C, 1], BF16)
            nc.vector.memset(ones_sb, 1.0)
            onesr_sb = res.tile([1, C], F32)
            nc.vector.memset(onesr_sb, 1.0)

            def chunk_tiles(prefix, dtype):
                return [res.tile([C, CHUNK], dtype, name=f"{prefix}{i}")
                        for i in range(NCH)]

            pose_t = chunk_tiles("pose", F32)
            posebf_t = chunk_tiles("posebf", BF16)
            idbf_t = chunk_tiles("idbf", BF16)
            q_t = chunk_tiles("q", BF16)
            k_t = chunk_tiles("k", BF16)
            v_t = chunk_tiles("v", BF16)
            vt_t = chunk_tiles("vt", BF16)   # [j (partition), jt*128 + c]

            # chunk-0 inputs first: they gate the pipeline start
            for ch in range(NCH):
                sl = slice(ch * CHUNK, (ch + 1) * CHUNK)
                nc.sync.dma_start(idbf_t[ch], idbf_d[:, sl])
                nc.sync.dma_start(posebf_t[ch], posebf_d[:, sl])
            for ch in range(NCH):
                sl = slice(ch * CHUNK, (ch + 1) * CHUNK)
                nc.gpsimd.dma_start(pose_t[ch], pose_d[:, sl])

            wqT = wt_sb[:, 0:C]
            wkT = wt_sb[:, C:2 * C]
            wvT = wt_sb[:, 2 * C:3 * C]

            # PE clock-gate pre-warm (no DMA deps)
            warm_sb = res.tile([C, 512], BF16)
            nc.vector.memset(warm_sb, 0.0)
            with tc.tile_pool(name="warm_ps", bufs=1, space="PSUM") as warm_ps:
                wp = warm_ps.tile([1, 512], F32)
                for _ in range(WARM):
                    nc.tensor.matmul(wp, lhsT=ones_sb, rhs=warm_sb,
                                     start=True, stop=True)

            # ---- chunk-0 QKV convs, full width (pool closed afterwards) ----
            with tc.tile_pool(name="conv_ps", bufs=3, space="PSUM") as conv_ps:
                kp = conv_ps.tile([C, CHUNK], F32, tag="cv", name="kp")
                qp = conv_ps.tile([C, CHUNK], F32, tag="cv", name="qp")
                vp = conv_ps.tile([C, CHUNK], F32, tag="cv", name="vp")
                for h in range(CHUNK // 512):
                    hs = slice(h * 512, (h + 1) * 512)
                    nc.tensor.matmul(kp[:, hs], lhsT=wkT,
                                     rhs=idbf_t[0][:, hs], start=True, stop=True)
                    nc.tensor.matmul(qp[:, hs], lhsT=wqT,
                                     rhs=posebf_t[0][:, hs], start=True, stop=True)
                    nc.tensor.matmul(vp[:, hs], lhsT=wvT,
                                     rhs=posebf_t[0][:, hs], start=True, stop=True)
                nc.scalar.activation(k_t[0], kp, AF.Identity, bias=bk_sb)
                nc.vector.tensor_scalar_add(q_t[0], qp, bq_sb)
                nc.scalar.copy(v_t[0], vp)
                nc.sync.dma_start_transpose(
                    vt_t[0].rearrange("p (t c) -> p t c", c=C), v_t[0])

            # ---- attention ----
            with (
                tc.tile_pool(name="et_ps", bufs=2, space="PSUM") as et_ps,
                tc.tile_pool(name="va_ps", bufs=1, space="PSUM") as va_ps,
                tc.tile_pool(name="z_ps", bufs=1, space="PSUM") as z_ps,
                tc.tile_pool(name="pt_sb", bufs=SKEW + QUAD + 2) as pt_pool,
                tc.tile_pool(name="qs_sb", bufs=2) as qs_pool,
                tc.tile_pool(name="nrm", bufs=2) as nrm,
                tc.tile_pool(name="outb", bufs=2) as outb,
                tc.tile_pool(name="dramp", bufs=2, space="DRAM") as dramp,
            ):
                # convs for chunks 1-3, interleaved into chunk-0's j-loop.
                # Each steals one tile of the et PSUM rotation; deadlines:
                # k_t[n] before chunk-0 E reaches jt=8n, vt_t[n] before the
                # lagged va reaches j-tile 8n (E jt=8n+SKEW), q_t[n] before
                # chunk n starts.
                CONV_JOBS = {
                    1: ("k", 1), 3: ("v", 1), 5: ("T", 1), 6: ("q", 1),
                    9: ("k", 2), 11: ("v", 2), 13: ("T", 2), 14: ("q", 2),
                    17: ("k", 3), 19: ("v", 3), 21: ("T", 3), 22: ("q", 3),
                }

                def emit_conv(part, ch):
                    if part == "T":
                        nc.sync.dma_start_transpose(
                            vt_t[ch].rearrange("p (t c) -> p t c", c=C),
                            v_t[ch])
                        return
                    cp = et_ps.tile([C, CHUNK], F32, tag="et", name="cp")
                    w, src = {"k": (wkT, idbf_t[ch]),
                              "q": (wqT, posebf_t[ch]),
                              "v": (wvT, posebf_t[ch])}[part]
                    for h in range(CHUNK // 512):
                        hs = slice(h * 512, (h + 1) * 512)
                        nc.tensor.matmul(cp[:, hs], lhsT=w, rhs=src[:, hs],
                                         start=True, stop=True)
                    if part == "k":
                        nc.scalar.activation(k_t[ch], cp, AF.Identity,
                                             bias=bk_sb)
                    elif part == "q":
                        nc.vector.tensor_scalar_add(q_t[ch], cp, bq_sb)
                    else:
                        nc.scalar.copy(v_t[ch], cp)

                for ch in range(NCH):
                    i0 = ch * CHUNK
                    isl = slice(i0, i0 + CHUNK)
                    last_ch = ch == NCH - 1
                    va = va_ps.tile([C, CHUNK], F32, name="va")
                    z = z_ps.tile([1, CHUNK], F32, tag="z", name="z")
                    # pose' = pose + gamma*bv (needed only at normalize)
                    nc.vector.tensor_scalar_add(pose_t[ch], pose_t[ch],
                                                bfin_sb)
                    pts = {}

                    def skew_at(j):
                        if not last_ch:
                            return SKEW
                        return max(2, min(SKEW, NJT + 2 - j))

                    lag_ptr = 0
                    n_steps = NJT + 2 if last_ch else NJT + SKEW
                    for jt in range(n_steps):
                        if ch == 0 and jt in CONV_JOBS:
                            part, cch = CONV_JOBS[jt]
                            emit_conv(part, cch)
                        if jt < NJT:
                            ksl = slice((jt % JPC) * 128, (jt % JPC + 1) * 128)
                            et = et_ps.tile([C, CHUNK], F32, tag="et",
                                            name="et")
                            for h in range(CHUNK // 512):
                                hs = slice(h * 512, (h + 1) * 512)
                                nc.tensor.matmul(
                                    et[:, hs], lhsT=k_t[jt // JPC][:, ksl],
                                    rhs=q_t[ch][:, hs], start=True, stop=True)
                            pt = pt_pool.tile([C, CHUNK], BF16, name="pt")
                            if _use_dve_exp(jt):
                                nc.vector.tensor_scalar(
                                    pt.bitcast(I16), et,
                                    SC_S, SC_B, ALU.mult, ALU.add)
                            else:
                                nc.scalar.activation(pt, et, AF.Exp)
                            pts[jt] = pt

                        while lag_ptr <= min(jt - skew_at(jt), NJT - 1):
                            lag = lag_ptr
                            lag_ptr += 1
                            vsl = slice((lag % JPC) * 128, (lag % JPC + 1) * 128)
                            pt = pts[lag]
                            for h in range(CHUNK // 512):
                                hs = slice(h * 512, (h + 1) * 512)
                                nc.tensor.matmul(
                                    va[:, hs], lhsT=vt_t[lag // JPC][:, vsl],
                                    rhs=pt[:, hs],
                                    start=(lag == 0),
                                    stop=(lag == NJT - 1))
                            # Z column sums (sum over j spans partitions ->
                            # M=1 matmuls over DVE pre-summed tiles).  Kernel
                            # tail: direct matmuls so Z doesn't wait on a
                            # serial DVE chain.
                            direct_z = last_ch and lag >= NJT - QUAD
                            if direct_z:
                                pts.pop(lag)
                                for h in range(CHUNK // 512):
                                    hs = slice(h * 512, (h + 1) * 512)
                                    nc.tensor.matmul(z[0:1, hs], lhsT=ones_sb,
                                                     rhs=pt[:, hs],
                                                     start=False,
                                                     stop=(lag == NJT - 1))
                            elif lag % QUAD == QUAD - 1:
                                qd = lag // QUAD
                                p0, p1, p2, p3 = (pts.pop(lag - 3),
                                                  pts.pop(lag - 2),
                                                  pts.pop(lag - 1),
                                                  pts.pop(lag))
                                sa = qs_pool.tile([C, CHUNK], BF16, tag="sa",
                                                  name="sa")
                                nc.vector.tensor_add(sa, p0, p1)
                                sb_ = qs_pool.tile([C, CHUNK], BF16, tag="sb",
                                                   name="sb_")
                                nc.vector.tensor_add(sb_, p2, p3)
                                sab = qs_pool.tile([C, CHUNK], BF16,
                                                   tag="sab", name="sab")
                                nc.vector.tensor_add(sab, sa, sb_)
                                # pair adjacent quad-sums so each M=1 Z
                                # matmul covers 8 j-tiles
                                last_q = (NJT // QUAD - 1) if not last_ch \
                                    else (NJT - QUAD) // QUAD - 1
                                if qd % 2 == 0 and qd != last_q:
                                    sab_prev = sab
                                    src = None
                                elif qd % 2 == 1:
                                    s8 = qs_pool.tile([C, CHUNK], BF16,
                                                      tag="s8", name="s8")
                                    nc.vector.tensor_add(s8, sab_prev, sab)
                                    src = s8
                                else:
                                    src = sab   # odd quad-count tail flush
                                if src is not None:
                                    z_stop = (not last_ch
                                              and qd == NJT // QUAD - 1)
                                    for h in range(CHUNK // 512):
                                        hs = slice(h * 512, (h + 1) * 512)
                                        nc.tensor.matmul(z[0:1, hs],
                                                         lhsT=ones_sb,
                                                         rhs=src[:, hs],
                                                         start=(qd <= 1),
                                                         stop=z_stop)

                    # free the va PSUM bank quickly, then normalize from SBUF
                    va_sb = nrm.tile([C, CHUNK], F32, tag="va_sb",
                                     name="va_sb")
                    nc.vector.tensor_copy(va_sb, va)
                    if not last_ch:
                        rz = nrm.tile([1, CHUNK], F32, tag="rz", name="rz")
                        nc.vector.reciprocal_approx_fast(rz, z)
                        # fold gamma into the tiny 1/Z row, then broadcast
                        # across partitions via a DRAM round-trip (latency
                        # hidden under the next chunk's compute)
                        nc.vector.tensor_scalar_mul(rz, rz, gam_sb[0:1, :])
                        zd = dramp.tile([1, CHUNK], F32, name="zd")
                        nc.sync.dma_start(zd, rz)
                        rzb = nrm.tile([C, CHUNK], F32, tag="rzb", name="rzb")
                        nc.sync.dma_start(rzb, zd.to_broadcast([C, CHUNK]))
                        t = nrm.tile([C, CHUNK], F32, tag="t", name="t")
                        nc.vector.tensor_mul(t, va_sb, rzb)
                        o = outb.tile([C, CHUNK], F32, name="o")
                        nc.vector.tensor_add(o, t, pose_t[ch])
                        nc.sync.dma_start(out_d[:, isl], o)
                    else:
                        # last chunk: broadcast gamma/Z on the (now idle) PE
                        # and drain in 512-wide half-pipelined steps
                        rz = nrm.tile([1, CHUNK], F32, tag="rz", name="rz")
                        rzb = et_ps.tile([C, CHUNK], F32, tag="et",
                                         name="rzb_ps")
                        t = nrm.tile([C, CHUNK], F32, tag="t", name="t")
                        o = outb.tile([C, CHUNK], F32, name="o")
                        for h in range(CHUNK // 512):
                            hs = slice(h * 512, (h + 1) * 512)
                            ihs = slice(i0 + h * 512, i0 + (h + 1) * 512)
                            nc.vector.reciprocal_approx_fast(
                                rz[0:1, hs], z[0:1, hs])
                            nc.vector.tensor_scalar_mul(
                                rz[0:1, hs], rz[0:1, hs], gam_sb[0:1, :])
                            nc.tensor.matmul(rzb[:, hs], lhsT=onesr_sb,
                                             rhs=rz[0:1, hs],
                                             start=True, stop=True)
                            nc.vector.tensor_mul(t[:, hs], va_sb[:, hs],
                                                 rzb[:, hs])
                            nc.vector.tensor_add(o[:, hs], t[:, hs],
                                                 pose_t[ch][:, hs])
                            nc.sync.dma_start(out_d[:, ihs], o[:, hs])

    nc.compile()
    return nc


def _get_nc():
    if "nc" not in _CACHE:
        _CACHE["nc"] = _build()
    return _CACHE["nc"]


def kernel(pose_f, id_f, Wq, bq, Wk, bk, Wv, bv, gamma, **run_kwargs):
    pose_f = np.asarray(pose_f, dtype=np.float32)
    id_f = np.asarray(id_f, dtype=np.float32)
    Wq = np.asarray(Wq, dtype=np.float32)
    Wk = np.asarray(Wk, dtype=np.float32)
    Wv = np.asarray(Wv, dtype=np.float32)
    bq = np.asarray(bq, dtype=np.float32)
    bk = np.asarray(bk, dtype=np.float32)
    bv = np.asarray(bv, dtype=np.float32)
    g = float(np.asarray(gamma, dtype=np.float32).reshape(-1)[0])

    bf = ml_dtypes.bfloat16
    wt = np.concatenate([Wq.T, Wk.T, Wv.T], axis=1).astype(bf)  # [C_in, 3C]
    posebf = pose_f.astype(bf)
    idbf = id_f.astype(bf)
    bq_c = np.ascontiguousarray(bq.reshape(C, 1))
    bk_c = np.ascontiguousarray(bk.reshape(C, 1))
    bfin = np.ascontiguousarray((g * bv).reshape(C, 1).astype(np.float32))
    gam = np.full((C, 1), g, dtype=np.float32)

    in_maps = []
    for b in range(B):
        in_maps.append({
            "pose": pose_f[b],
            "posebf": posebf[b],
            "idbf": idbf[b],
            "wt": wt,
            "bq": bq_c,
            "bk": bk_c,
            "bfin": bfin,
            "gam": gam,
        })

    nc = _get_nc()
    res = run_bass_kernel_spmd(nc, in_maps, core_ids=list(range(B)), **run_kwargs)
    out = np.stack([res.results[b]["out"] for b in range(B)], axis=0)
    if run_kwargs:
        _CACHE["last_result"] = res
    return out
